# revision 1
# baseline (speedup 1.0000x reference)
"""Single-head cross-attention (layernorm + QKV proj + softmax(QK^T)V) on 8 NeuronCores.

Sharding: data-parallel over batch B=8, one batch element per core.

Per-core device program (all matmuls bf16 with fp32 PSUM accumulation):
  1. Layernorm (no affine; g/b folded into weights on host) of target/source_k/source_v
     in natural [token, d] layout via bn_stats, output bf16.
  2. DMA-xbar transpose of normalized activations to [d, token] layout.
  3. Projections with pre-transposed weights produce qT/kT in [e, token] layout and
     v in natural [token, e] layout (so no further transposes are needed).
  4. scores^T[j, i] = kT^T qT, exp via ScalarE (scale 1/sqrt(d) fused, no max
     subtraction: |scores*scale| < 3), giving unnormalized attn^T in bf16.
  5. out_u[i, e] = attn^T.T @ v and Z[i] = attn^T.T @ ones accumulate in PSUM;
     out = out_u / Z.
"""

import os
from contextlib import ExitStack

import numpy as np
import ml_dtypes

import concourse.bass as bass
import concourse.bacc as bacc
import concourse.mybir as mybir
import concourse.tile as tile
from concourse.bass import ts, ds
from concourse.bass_utils import run_bass_kernel_spmd

BF16 = mybir.dt.bfloat16
F32 = mybir.dt.float32

B, T, D = 8, 2048, 1024
EPS = 1e-5
SCALE = float(D) ** -0.5
P = 128
N_IT = T // P          # 16 token tiles of 128
N_DB = D // P          # 8 d-blocks of 128
N_EB = D // P          # 8 e-blocks of 128
N_IC = T // 512        # 4 token chunks of 512
N_EC = D // 512        # 2 e chunks of 512


def _ln_transpose(nc, pool_stage, streams, eps_t):
    """Layernorm + DMA-xbar transpose for one or more tensors, interleaved.

    streams: list of (name, x_dram, xt_tile). Interleaving independent chains
    keeps the in-order DVE queue busy while each chain crosses to ACT for the
    fused 1/sqrt(var+eps) and back. Software-pipelined by one tile.
    """
    tiles = {}

    def stats_stage(s, it):
        name, tb, x_dram, xt_tile = streams[s]
        x_raw = pool_stage.tile([P, D], F32, tag=f"x_raw_{tb}", bufs=2,
                                name=f"x_raw_{name}_{it}")
        nc.gpsimd.dma_start(out=x_raw, in_=x_dram[ts(it, P), :])
        stats = pool_stage.tile([P, 2, 6], F32, tag=f"stats_{tb}", bufs=3,
                                name=f"stats_{name}_{it}")
        for sb in range(2):
            nc.vector.bn_stats(out=stats[:, sb, :], in_=x_raw[:, ts(sb, 512)])
        mv = pool_stage.tile([P, 2], F32, tag=f"mv_{tb}", bufs=3, name=f"mv_{name}_{it}")
        nc.vector.bn_aggr(out=mv, in_=stats)
        rstd = pool_stage.tile([P, 1], F32, tag=f"rstd_{tb}", bufs=3,
                               name=f"rstd_{name}_{it}")
        nc.scalar.activation(
            out=rstd, in_=mv[:, 1:2],
            func=mybir.ActivationFunctionType.Abs_reciprocal_sqrt,
            bias=eps_t,
        )
        tiles[(s, it)] = (x_raw, mv, rstd)

    def apply_stage(s, it):
        name, tb, x_dram, xt_tile = streams[s]
        x_raw, mv, rstd = tiles.pop((s, it))
        ln_out = pool_stage.tile([P, D], BF16, tag=f"ln_out_{tb}", bufs=2,
                                 name=f"ln_out_{name}_{it}")
        nc.vector.tensor_scalar(
            out=ln_out, in0=x_raw, scalar1=mv[:, 0:1], scalar2=rstd,
            op0=mybir.AluOpType.subtract, op1=mybir.AluOpType.mult,
        )
        nc.sync.dma_start(out=xt_tile[:, it], in_=ln_out, transpose=True)

    ns = len(streams)
    for it in range(N_IT):
        for s in range(ns):
            stats_stage(s, it)
        if it > 0:
            for s in range(ns):
                apply_stage(s, it - 1)
    for s in range(ns):
        apply_stage(s, N_IT - 1)


def _xt_rhs(xt_tile, db, ic):
    """Moving operand [d-block partition, 512 tokens] for token chunk ic."""
    return xt_tile[:, ds(ic * 4, 4), db, :]


def build_module() -> bass.Bass:
    nc = bacc.Bacc("TRN2", target_bir_lowering=False)

    x_t = nc.dram_tensor("x_t", [T, D], F32, kind="ExternalInput")
    x_k = nc.dram_tensor("x_k", [T, D], F32, kind="ExternalInput")
    x_v = nc.dram_tensor("x_v", [T, D], F32, kind="ExternalInput")
    wq = nc.dram_tensor("wq", [D, D], BF16, kind="ExternalInput")  # pre-transposed [d, e]
    wk = nc.dram_tensor("wk", [D, D], BF16, kind="ExternalInput")
    wv = nc.dram_tensor("wv", [D, D], BF16, kind="ExternalInput")
    bq = nc.dram_tensor("bq", [D], F32, kind="ExternalInput")
    bk = nc.dram_tensor("bk", [D], F32, kind="ExternalInput")
    bv = nc.dram_tensor("bv", [D], F32, kind="ExternalInput")
    out = nc.dram_tensor("out", [T, D], F32, kind="ExternalOutput")

    with tile.TileContext(nc) as tc, ExitStack() as ctx:
        const = ctx.enter_context(tc.tile_pool(name="const", bufs=1))
        qkv = ctx.enter_context(tc.tile_pool(name="qkv", bufs=1))
        mm_ps = ctx.enter_context(tc.tile_pool(name="mm_ps", bufs=4, space="PSUM"))

        # ---- constants ----
        eps_t = const.tile([P, 1], F32)
        nc.vector.memset(eps_t, EPS)
        ones_t = const.tile([P, 1], BF16)
        nc.vector.memset(ones_t, 1.0)
        bq_sb = const.tile([P, N_EB], F32)
        nc.gpsimd.dma_start(out=bq_sb, in_=bq[:].rearrange("(a p) -> p a", p=P))
        bk_sb = const.tile([P, N_EB], F32)
        nc.gpsimd.dma_start(out=bk_sb, in_=bk[:].rearrange("(a p) -> p a", p=P))
        bv_ap = bv[:]
        bv_bc = const.tile([P, D], F32)
        nc.gpsimd.dma_start(
            out=bv_bc,
            in_=bass.AP(tensor=bv_ap.tensor, offset=bv_ap.offset,
                        ap=[[0, P]] + list(bv_ap.ap)),
        )

        # ---- persistent projection outputs ----
        qT = qkv.tile([P, N_EB, T], BF16)        # qT[p, eb, i] = q[i, eb*128+p]
        kT = qkv.tile([P, N_EB, T], BF16)
        v_sb = qkv.tile([P, N_IT, D], BF16)      # v[p, jt, e] = v[jt*128+p, e]

        with tc.tile_pool(name="proj_phase", bufs=1) as pp:
            def projectT(w_dram, bias_sb, xt_tile, dstT, pname):
                """dstT[p, eb, i] = sum_d ln[i, d] * w[d, eb*128+p] + bias."""
                for ic in range(N_IC):
                    for eb in range(N_EB):
                        w_sl = pp.tile([P, N_DB, P], BF16, tag="w_sl", bufs=3,
                                       name=f"w_{pname}_{ic}_{eb}")
                        nc.sync.dma_start(
                            out=w_sl,
                            in_=w_dram[:, ts(eb, P)].rearrange("(a p) e -> p a e", p=P),
                        )
                        ps = mm_ps.tile([P, 512], F32, tag="mm", name=f"ps_{pname}_{eb}_{ic}")
                        for db in range(N_DB):
                            nc.tensor.matmul(
                                ps, lhsT=w_sl[:, db, :],
                                rhs=_xt_rhs(xt_tile, db, ic),
                                start=(db == 0), stop=(db == N_DB - 1),
                            )
                        nc.scalar.activation(
                            out=dstT[:, eb, ts(ic, 512)], in_=ps,
                            func=mybir.ActivationFunctionType.Identity,
                            bias=bias_sb[:, eb:eb + 1],
                        )

            # target -> qT
            with nc.named_scope("ln_t"):
                xt_t = pp.tile([P, N_IT, N_DB, P], BF16, tag="xt", bufs=2)
                _ln_transpose(nc, pp, [("t", "a", x_t, xt_t)], eps_t)
            with nc.named_scope("proj_q"):
                projectT(wq, bq_sb, xt_t, qT, "q")

            # source_k -> kT
            with nc.named_scope("ln_k"):
                xt_k = pp.tile([P, N_IT, N_DB, P], BF16, tag="xt", bufs=2)
                _ln_transpose(nc, pp, [("k", "b", x_k, xt_k)], eps_t)
            with nc.named_scope("proj_k"):
                projectT(wk, bk_sb, xt_k, kT, "k")

            # source_v -> v (natural layout): v[jt, e] = sum_d ln_v[j, d] w_v[d, e] + bv
            with nc.named_scope("ln_v"):
                xt_v = pp.tile([P, N_IT, N_DB, P], BF16, tag="xt", bufs=2)
                _ln_transpose(nc, pp, [("v", "a", x_v, xt_v)], eps_t)
            with nc.named_scope("proj_v"):
                for ec in range(N_EC):
                    w_ec = pp.tile([P, N_DB, 512], BF16, tag="wv_keep", bufs=1,
                                   name=f"wv_{ec}")
                    nc.sync.dma_start(
                        out=w_ec,
                        in_=wv[:, ts(ec, 512)].rearrange("(a p) e -> p a e", p=P),
                    )
                    for jt in range(N_IT):
                        ps = mm_ps.tile([P, 512], F32, tag="mm", name=f"ps_v_{jt}_{ec}")
                        for db in range(N_DB):
                            nc.tensor.matmul(
                                ps,
                                lhsT=xt_v[:, jt, db, :],
                                rhs=w_ec[:, db, :],
                                start=(db == 0), stop=(db == N_DB - 1),
                            )
                        nc.vector.tensor_add(
                            out=v_sb[:, jt, ts(ec, 512)], in0=ps,
                            in1=bv_bc[:, ts(ec, 512)],
                        )

        # ---- attention ----
        attv_ps = ctx.enter_context(tc.tile_pool(name="attv_ps", bufs=2, space="PSUM"))
        with tc.tile_pool(name="att", bufs=1) as att:
            for ic in range(N_IC):
                with nc.named_scope(f"scores_{ic}"):
                    aT = att.tile([P, N_IT, 512], BF16, tag="aT", bufs=2,
                                  name=f"aT_{ic}")
                    for jt in range(N_IT):
                        ps = mm_ps.tile([P, 512], F32, tag="mm", name=f"ps_s_{ic}_{jt}")
                        for eb in range(N_EB):
                            nc.tensor.matmul(
                                ps, lhsT=kT[:, eb, ts(jt, P)],
                                rhs=qT[:, eb, ts(ic, 512)],
                                start=(eb == 0), stop=(eb == N_EB - 1),
                            )
                        nc.scalar.activation(
                            out=aT[:, jt, :], in_=ps,
                            func=mybir.ActivationFunctionType.Exp, scale=SCALE,
                        )
                with nc.named_scope(f"attv_{ic}"):
                    for isub in range(4):
                        ou = attv_ps.tile([P, D], F32, tag="ou", name=f"ou_{ic}_{isub}")
                        zz = mm_ps.tile([P, 1], F32, tag="mm", name=f"z_{ic}_{isub}")
                        # same-bank runs of 16 accumulating matmuls (bank cycling
                        # between consecutive matmuls forces PE micro-stalls)
                        for ec in range(N_EC):
                            for jt in range(N_IT):
                                nc.tensor.matmul(
                                    ou[:, ts(ec, 512)], lhsT=aT[:, jt, ts(isub, P)],
                                    rhs=v_sb[:, jt, ts(ec, 512)],
                                    start=(jt == 0), stop=(jt == N_IT - 1))
                        for jt in range(N_IT):
                            nc.tensor.matmul(zz, lhsT=aT[:, jt, ts(isub, P)], rhs=ones_t,
                                             start=(jt == 0), stop=(jt == N_IT - 1))
                        rz = att.tile([P, 1], F32, tag="rz", bufs=2,
                                      name=f"rz_{ic}_{isub}")
                        nc.vector.reciprocal(out=rz, in_=zz)
                        o_sb = att.tile([P, D], F32, tag="o_sb", bufs=2,
                                        name=f"o_{ic}_{isub}")
                        nc.vector.tensor_scalar_mul(out=o_sb, in0=ou, scalar1=rz)
                        nc.sync.dma_start(out=out[ts(ic * 4 + isub, P), :], in_=o_sb)

    nc.compile()
    return nc


_NC_CACHE = None


def _get_module():
    global _NC_CACHE
    if _NC_CACHE is None:
        _NC_CACHE = build_module()
    return _NC_CACHE


def kernel(target, source_k, source_v, Wq, bq, Wk, bk, Wv, bv,
           g_t, b_t, g_k, b_k, g_v, b_v):
    target = np.asarray(target, dtype=np.float32)
    source_k = np.asarray(source_k, dtype=np.float32)
    source_v = np.asarray(source_v, dtype=np.float32)
    Wq = np.asarray(Wq, dtype=np.float32); bq = np.asarray(bq, dtype=np.float32)
    Wk = np.asarray(Wk, dtype=np.float32); bk = np.asarray(bk, dtype=np.float32)
    Wv = np.asarray(Wv, dtype=np.float32); bv = np.asarray(bv, dtype=np.float32)
    g_t = np.asarray(g_t, dtype=np.float32); b_t = np.asarray(b_t, dtype=np.float32)
    g_k = np.asarray(g_k, dtype=np.float32); b_k = np.asarray(b_k, dtype=np.float32)
    g_v = np.asarray(g_v, dtype=np.float32); b_v = np.asarray(b_v, dtype=np.float32)

    bf16 = ml_dtypes.bfloat16
    # Fold the layernorm affine (g, b) into the projection weights/biases:
    #   LN_affine(x) @ W.T + b  ==  LN_plain(x) @ (W*g).T + (b + W @ b_ln)
    wqT = np.ascontiguousarray((Wq * g_t[None, :]).T).astype(bf16)
    wkT = np.ascontiguousarray((Wk * g_k[None, :]).T).astype(bf16)
    wvT = np.ascontiguousarray((Wv * g_v[None, :]).T).astype(bf16)
    bq_f = bq + Wq @ b_t
    bk_f = bk + Wk @ b_k
    bv_f = bv + Wv @ b_v

    nc = _get_module()
    in_maps = []
    for b in range(B):
        in_maps.append({
            "x_t": np.ascontiguousarray(target[b]),
            "x_k": np.ascontiguousarray(source_k[b]),
            "x_v": np.ascontiguousarray(source_v[b]),
            "wq": wqT, "wk": wkT, "wv": wvT,
            "bq": bq_f, "bk": bk_f, "bv": bv_f,
        })

    res = run_bass_kernel_spmd(nc, in_maps, core_ids=list(range(B)),
                               trace=bool(int(os.environ.get("KERNEL_TRACE", "0"))))
    out = np.stack([res.results[b]["out"] for b in range(B)], axis=0)
    kernel.last_results = res
    return out



# revision 3
# speedup vs baseline: 1.0673x; 1.0673x over previous
"""Single-head cross-attention (layernorm + QKV proj + softmax(QK^T)V) on 8 NeuronCores.

Sharding: data-parallel over batch B=8, one batch element per core.

Per-core device program (all matmuls bf16 with fp32 PSUM accumulation), structured
to keep the PE array streaming back-to-back (no DMA-wait stalls, HAM stays warm):

  - All three projection weights are loaded into SBUF once at kernel start with
    partition-contiguous 2KB descriptors (the dominant baseline stall was
    LDWEIGHTS waiting on per-(ic,eb) strided weight DMAs).
  - Each input tensor is processed in 512-token chunks, software-pipelined:
    LN chunk ic+1 (DVE) runs under the projection matmuls of chunk ic (PE).
    LN input is loaded as bf16 (SWDGE cast) for 2x DVE rate; normalized output
    is DMA-xbar transposed to [d, token] layout for the projections.
  - Projections produce qT/kT in [e, token] layout and v in natural [token, e]
    layout, so attention needs no further transposes.
  - scores^T[j, i] = kT^T qT, exp via ScalarE (scale 1/sqrt(d) fused, no max
    subtraction: |scores*scale| < 3), giving unnormalized attn^T in bf16.
  - out_u[i, e] = attn^T.T @ v and Z[i] = attn^T.T @ ones accumulate in PSUM;
    out = out_u / Z. The attention loop is software-pipelined one chunk deep
    (issue order S0, S1, A0, S2, A1, ...) so attv never waits on exp.
"""

import os
from contextlib import ExitStack

import numpy as np
import ml_dtypes

import concourse.bass as bass
import concourse.bacc as bacc
import concourse.mybir as mybir
import concourse.tile as tile
from concourse.bass import ts, ds
from concourse.bass_utils import run_bass_kernel_spmd

BF16 = mybir.dt.bfloat16
F32 = mybir.dt.float32

B, T, D = 8, 2048, 1024
EPS = 1e-5
SCALE = float(D) ** -0.5
P = 128
N_IT = T // P          # 16 token tiles of 128
N_DB = D // P          # 8 d-blocks of 128
N_EB = D // P          # 8 e-blocks of 128
N_IC = T // 512        # 4 token chunks of 512
N_EC = D // 512        # 2 e chunks of 512
CT = 4                 # token tiles per chunk


def build_module() -> bass.Bass:
    nc = bacc.Bacc("TRN2", target_bir_lowering=False)

    x_t = nc.dram_tensor("x_t", [T, D], F32, kind="ExternalInput")
    x_k = nc.dram_tensor("x_k", [T, D], F32, kind="ExternalInput")
    x_v = nc.dram_tensor("x_v", [T, D], F32, kind="ExternalInput")
    wq = nc.dram_tensor("wq", [D, D], BF16, kind="ExternalInput")  # pre-transposed [d, e]
    wk = nc.dram_tensor("wk", [D, D], BF16, kind="ExternalInput")
    wv = nc.dram_tensor("wv", [D, D], BF16, kind="ExternalInput")
    bq = nc.dram_tensor("bq", [D], F32, kind="ExternalInput")
    bk = nc.dram_tensor("bk", [D], F32, kind="ExternalInput")
    bv = nc.dram_tensor("bv", [D], F32, kind="ExternalInput")
    out = nc.dram_tensor("out", [T, D], F32, kind="ExternalOutput")

    with tile.TileContext(nc) as tc, ExitStack() as ctx:
        const = ctx.enter_context(tc.tile_pool(name="const", bufs=1))
        qkv = ctx.enter_context(tc.tile_pool(name="qkv", bufs=1))
        mm_ps = ctx.enter_context(tc.tile_pool(name="mm_ps", bufs=4, space="PSUM"))

        # ---- constants ----
        eps_t = const.tile([P, 1], F32)
        nc.vector.memset(eps_t, EPS)
        ones_t = const.tile([P, 1], BF16)
        nc.vector.memset(ones_t, 1.0)
        bq_sb = const.tile([P, N_EB], F32)
        nc.gpsimd.dma_start(out=bq_sb, in_=bq[:].rearrange("(a p) -> p a", p=P))
        bk_sb = const.tile([P, N_EB], F32)
        nc.gpsimd.dma_start(out=bk_sb, in_=bk[:].rearrange("(a p) -> p a", p=P))
        bv_ap = bv[:]
        bv_bc = const.tile([P, D], F32)
        nc.gpsimd.dma_start(
            out=bv_bc,
            in_=bass.AP(tensor=bv_ap.tensor, offset=bv_ap.offset,
                        ap=[[0, P]] + list(bv_ap.ap)),
        )

        # ---- persistent projection outputs ----
        qT = qkv.tile([P, N_EB, T], BF16)        # qT[p, eb, i] = q[i, eb*128+p]
        kT = qkv.tile([P, N_EB, T], BF16)
        v_sb = qkv.tile([P, N_IT, D], BF16)      # v[p, jt, e] = v[jt*128+p, e]

        with tc.tile_pool(name="proj_phase", bufs=1) as pp:
            # ---- resident weights: w_sb[p, a, e] = w[a*128+p, e] ----
            # (2KB contiguous per descriptor row -> full DMA line rate)
            w_sbs = {}
            for name, w_dram, queue in (("q", wq, nc.scalar),
                                        ("k", wk, nc.scalar),
                                        ("v", wv, nc.gpsimd)):
                w_sbs[name] = pp.tile([P, N_DB, D], BF16, tag=f"w_{name}", bufs=1,
                                      name=f"w_sb_{name}")
                queue.dma_start(out=w_sbs[name],
                                in_=w_dram[:, :].rearrange("(a p) e -> p a e", p=P))

            def ln_chunk(pname, x_dram, xt_tile, ic):
                """Layernorm + transpose of one 512-token chunk into xt_tile."""
                held = []
                for s in range(CT):
                    it = ic * CT + s
                    x_raw = pp.tile([P, D], BF16, tag="x_raw", bufs=3,
                                    name=f"x_raw_{pname}_{it}")
                    nc.gpsimd.dma_start(out=x_raw, in_=x_dram[ts(it, P), :])
                    stats = pp.tile([P, 2, 6], F32, tag="stats", bufs=3,
                                    name=f"stats_{pname}_{it}")
                    for sb in range(2):
                        nc.vector.bn_stats(out=stats[:, sb, :], in_=x_raw[:, ts(sb, 512)])
                    mv = pp.tile([P, 2], F32, tag="mv", bufs=3, name=f"mv_{pname}_{it}")
                    nc.vector.bn_aggr(out=mv, in_=stats)
                    rstd = pp.tile([P, 1], F32, tag="rstd", bufs=3,
                                   name=f"rstd_{pname}_{it}")
                    nc.scalar.activation(
                        out=rstd, in_=mv[:, 1:2],
                        func=mybir.ActivationFunctionType.Abs_reciprocal_sqrt,
                        bias=eps_t,
                    )
                    held.append((s, x_raw, mv, rstd))
                for s, x_raw, mv, rstd in held:
                    ln_out = pp.tile([P, D], BF16, tag="ln_out", bufs=2,
                                     name=f"ln_out_{pname}_{ic}_{s}")
                    nc.vector.tensor_scalar(
                        out=ln_out, in0=x_raw, scalar1=mv[:, 0:1], scalar2=rstd,
                        op0=mybir.AluOpType.subtract, op1=mybir.AluOpType.mult,
                    )
                    nc.sync.dma_start(out=xt_tile[:, s], in_=ln_out, transpose=True)

            def proj_chunkT(w_sb, bias_sb, xt_tile, dstT, pname, ic):
                """dstT[:, eb, ic*512:+512] = w^T @ ln_chunk + bias."""
                for eb in range(N_EB):
                    ps = mm_ps.tile([P, 512], F32, tag="mm", name=f"ps_{pname}_{eb}_{ic}")
                    for db in range(N_DB):
                        nc.tensor.matmul(
                            ps, lhsT=w_sb[:, db, ts(eb, P)],
                            rhs=xt_tile[:, :, db, :],
                            start=(db == 0), stop=(db == N_DB - 1),
                        )
                    nc.scalar.activation(
                        out=dstT[:, eb, ts(ic, 512)], in_=ps,
                        func=mybir.ActivationFunctionType.Identity,
                        bias=bias_sb[:, eb:eb + 1],
                    )

            def proj_chunkV(w_sb, xt_tile, ic):
                """v_sb[:, ic*4+s, :] = ln_chunk^T @ w + bias (natural layout)."""
                for s in range(CT):
                    jt = ic * CT + s
                    for ec in range(N_EC):
                        ps = mm_ps.tile([P, 512], F32, tag="mm",
                                        name=f"ps_v_{jt}_{ec}")
                        for db in range(N_DB):
                            nc.tensor.matmul(
                                ps,
                                lhsT=xt_tile[:, s, db, :],
                                rhs=w_sb[:, db, ts(ec, 512)],
                                start=(db == 0), stop=(db == N_DB - 1),
                            )
                        nc.vector.tensor_add(
                            out=v_sb[:, jt, ts(ec, 512)], in0=ps,
                            in1=bv_bc[:, ts(ec, 512)],
                        )

            # ---- three phases, each software-pipelined one chunk deep ----
            phases = [
                ("t", x_t, lambda xt, ic: proj_chunkT(w_sbs["q"], bq_sb, xt, qT, "q", ic)),
                ("k", x_k, lambda xt, ic: proj_chunkT(w_sbs["k"], bk_sb, xt, kT, "k", ic)),
                ("v", x_v, lambda xt, ic: proj_chunkV(w_sbs["v"], xt, ic)),
            ]
            for pname, x_dram, proj_fn in phases:
                xts = {}
                with nc.named_scope(f"phase_{pname}"):
                    for ic in range(N_IC + 1):
                        if ic < N_IC:
                            xts[ic] = pp.tile([P, CT, N_DB, P], BF16, tag="xt",
                                              bufs=2, name=f"xt_{pname}_{ic}")
                            ln_chunk(pname, x_dram, xts[ic], ic)
                        if ic > 0:
                            proj_fn(xts.pop(ic - 1), ic - 1)

        # ---- attention, software-pipelined one chunk deep ----
        attv_ps = ctx.enter_context(tc.tile_pool(name="attv_ps", bufs=2, space="PSUM"))
        with tc.tile_pool(name="att", bufs=1) as att:
            aTs = {}

            def scores_chunk(ic):
                aT = att.tile([P, N_IT, 512], BF16, tag="aT", bufs=2, name=f"aT_{ic}")
                aTs[ic] = aT
                with nc.named_scope(f"scores_{ic}"):
                    for jt in range(N_IT):
                        ps = mm_ps.tile([P, 512], F32, tag="mm", name=f"ps_s_{ic}_{jt}")
                        for eb in range(N_EB):
                            nc.tensor.matmul(
                                ps, lhsT=kT[:, eb, ts(jt, P)],
                                rhs=qT[:, eb, ts(ic, 512)],
                                start=(eb == 0), stop=(eb == N_EB - 1),
                            )
                        nc.scalar.activation(
                            out=aT[:, jt, :], in_=ps,
                            func=mybir.ActivationFunctionType.Exp, scale=SCALE,
                        )

            def attv_chunk(ic):
                aT = aTs.pop(ic)
                with nc.named_scope(f"attv_{ic}"):
                    for isub in range(4):
                        ou = attv_ps.tile([P, D], F32, tag="ou", name=f"ou_{ic}_{isub}")
                        zz = mm_ps.tile([P, 1], F32, tag="mm", name=f"z_{ic}_{isub}")
                        # same-bank runs of 16 accumulating matmuls (bank cycling
                        # between consecutive matmuls forces PE micro-stalls)
                        for ec in range(N_EC):
                            for jt in range(N_IT):
                                nc.tensor.matmul(
                                    ou[:, ts(ec, 512)], lhsT=aT[:, jt, ts(isub, P)],
                                    rhs=v_sb[:, jt, ts(ec, 512)],
                                    start=(jt == 0), stop=(jt == N_IT - 1))
                        for jt in range(N_IT):
                            nc.tensor.matmul(zz, lhsT=aT[:, jt, ts(isub, P)], rhs=ones_t,
                                             start=(jt == 0), stop=(jt == N_IT - 1))
                        rz = att.tile([P, 1], F32, tag="rz", bufs=2,
                                      name=f"rz_{ic}_{isub}")
                        nc.vector.reciprocal(out=rz, in_=zz)
                        o_sb = att.tile([P, D], F32, tag="o_sb", bufs=2,
                                        name=f"o_{ic}_{isub}")
                        nc.vector.tensor_scalar_mul(out=o_sb, in0=ou, scalar1=rz)
                        nc.sync.dma_start(out=out[ts(ic * 4 + isub, P), :], in_=o_sb)

            scores_chunk(0)
            for ic in range(N_IC):
                if ic + 1 < N_IC:
                    scores_chunk(ic + 1)
                attv_chunk(ic)

    nc.compile()
    return nc


_NC_CACHE = None


def _get_module():
    global _NC_CACHE
    if _NC_CACHE is None:
        _NC_CACHE = build_module()
    return _NC_CACHE


def kernel(target, source_k, source_v, Wq, bq, Wk, bk, Wv, bv,
           g_t, b_t, g_k, b_k, g_v, b_v):
    target = np.asarray(target, dtype=np.float32)
    source_k = np.asarray(source_k, dtype=np.float32)
    source_v = np.asarray(source_v, dtype=np.float32)
    Wq = np.asarray(Wq, dtype=np.float32); bq = np.asarray(bq, dtype=np.float32)
    Wk = np.asarray(Wk, dtype=np.float32); bk = np.asarray(bk, dtype=np.float32)
    Wv = np.asarray(Wv, dtype=np.float32); bv = np.asarray(bv, dtype=np.float32)
    g_t = np.asarray(g_t, dtype=np.float32); b_t = np.asarray(b_t, dtype=np.float32)
    g_k = np.asarray(g_k, dtype=np.float32); b_k = np.asarray(b_k, dtype=np.float32)
    g_v = np.asarray(g_v, dtype=np.float32); b_v = np.asarray(b_v, dtype=np.float32)

    bf16 = ml_dtypes.bfloat16
    # Fold the layernorm affine (g, b) into the projection weights/biases:
    #   LN_affine(x) @ W.T + b  ==  LN_plain(x) @ (W*g).T + (b + W @ b_ln)
    wqT = np.ascontiguousarray((Wq * g_t[None, :]).T).astype(bf16)
    wkT = np.ascontiguousarray((Wk * g_k[None, :]).T).astype(bf16)
    wvT = np.ascontiguousarray((Wv * g_v[None, :]).T).astype(bf16)
    bq_f = bq + Wq @ b_t
    bk_f = bk + Wk @ b_k
    bv_f = bv + Wv @ b_v

    nc = _get_module()
    in_maps = []
    for b in range(B):
        in_maps.append({
            "x_t": np.ascontiguousarray(target[b]),
            "x_k": np.ascontiguousarray(source_k[b]),
            "x_v": np.ascontiguousarray(source_v[b]),
            "wq": wqT, "wk": wkT, "wv": wvT,
            "bq": bq_f, "bk": bk_f, "bv": bv_f,
        })

    res = run_bass_kernel_spmd(nc, in_maps, core_ids=list(range(B)),
                               trace=bool(int(os.environ.get("KERNEL_TRACE", "0"))))
    out = np.stack([res.results[b]["out"] for b in range(B)], axis=0)
    kernel.last_results = res
    return out


# revision 6
# speedup vs baseline: 1.2293x; 1.1518x over previous
"""Single-head cross-attention (layernorm + QKV proj + softmax(QK^T)V) on 8 NeuronCores.

Sharding: data-parallel over batch B=8, one batch element per core.

Per-core device program (all matmuls bf16 with fp32 PSUM accumulation), structured
to keep the PE array streaming back-to-back (no DMA-wait stalls, HAM stays warm):

  - All three projection weights are loaded into SBUF once at kernel start with
    partition-contiguous 2KB descriptors (the dominant baseline stall was
    LDWEIGHTS waiting on per-(ic,eb) strided weight DMAs).
  - Each input tensor is processed in 512-token chunks, software-pipelined:
    LN chunk ic+1 (DVE) runs under the projection matmuls of chunk ic (PE).
    LN input is loaded as bf16 (SWDGE cast) for 2x DVE rate; normalized output
    is DMA-xbar transposed to [d, token] layout for the projections.
  - Projections produce qT/kT in [e, token] layout and v in natural [token, e]
    layout, so attention needs no further transposes.
  - scores^T[j, i] = kT^T qT, exp via ScalarE (scale 1/sqrt(d) fused, no max
    subtraction: |scores*scale| < 3), giving unnormalized attn^T in bf16.
  - out_u[i, e] = attn^T.T @ v and Z[i] = attn^T.T @ ones accumulate in PSUM;
    out = out_u / Z. The attention loop is software-pipelined one chunk deep
    (issue order S0, S1, A0, S2, A1, ...) so attv never waits on exp.
"""

import os
from contextlib import ExitStack

import numpy as np
import ml_dtypes

import concourse.bass as bass
import concourse.bacc as bacc
import concourse.mybir as mybir
import concourse.tile as tile
from concourse.bass import ts, ds
from concourse.bass_utils import run_bass_kernel_spmd

BF16 = mybir.dt.bfloat16
F32 = mybir.dt.float32

B, T, D = 8, 2048, 1024
EPS = 1e-5
SCALE = float(D) ** -0.5
P = 128
N_IT = T // P          # 16 token tiles of 128
N_DB = D // P          # 8 d-blocks of 128
N_EB = D // P          # 8 e-blocks of 128
N_IC = T // 512        # 4 token chunks of 512
N_EC = D // 512        # 2 e chunks of 512
CT = 4                 # token tiles per chunk


def build_module() -> bass.Bass:
    nc = bacc.Bacc("TRN2", target_bir_lowering=False)

    x_t = nc.dram_tensor("x_t", [T, D], F32, kind="ExternalInput")
    x_k = nc.dram_tensor("x_k", [T, D], F32, kind="ExternalInput")
    x_v = nc.dram_tensor("x_v", [T, D], F32, kind="ExternalInput")
    wq = nc.dram_tensor("wq", [D, D], BF16, kind="ExternalInput")  # pre-transposed [d, e]
    wk = nc.dram_tensor("wk", [D, D], BF16, kind="ExternalInput")
    wv = nc.dram_tensor("wv", [D, D], BF16, kind="ExternalInput")
    bq = nc.dram_tensor("bq", [D], F32, kind="ExternalInput")
    bk = nc.dram_tensor("bk", [D], F32, kind="ExternalInput")
    bv = nc.dram_tensor("bv", [D], F32, kind="ExternalInput")
    out = nc.dram_tensor("out", [T, D], F32, kind="ExternalOutput")

    with tile.TileContext(nc) as tc, ExitStack() as ctx:
        const = ctx.enter_context(tc.tile_pool(name="const", bufs=1))
        qkv = ctx.enter_context(tc.tile_pool(name="qkv", bufs=1))
        mm_ps = ctx.enter_context(tc.tile_pool(name="mm_ps", bufs=4, space="PSUM"))

        # ---- constants (DMAs issued in the prologue below, after the
        # critical first x-chunk + wq loads) ----
        eps_t = const.tile([P, 1], F32)
        nc.vector.memset(eps_t, EPS)
        ones_t = const.tile([P, 1], BF16)
        nc.vector.memset(ones_t, 1.0)
        bq_sb = const.tile([P, N_EB], F32)
        bk_sb = const.tile([P, N_EB], F32)
        bv_bc = const.tile([P, D], F32)

        def load_consts():
            nc.gpsimd.dma_start(out=bq_sb, in_=bq[:].rearrange("(a p) -> p a", p=P))
            nc.gpsimd.dma_start(out=bk_sb, in_=bk[:].rearrange("(a p) -> p a", p=P))
            bv_ap = bv[:]
            nc.gpsimd.dma_start(
                out=bv_bc,
                in_=bass.AP(tensor=bv_ap.tensor, offset=bv_ap.offset,
                            ap=[[0, P]] + list(bv_ap.ap)),
            )

        # ---- persistent projection outputs ----
        qT = qkv.tile([P, N_EB, T], BF16)        # qT[p, eb, i] = q[i, eb*128+p]
        kT = qkv.tile([P, N_EB, T], BF16)
        v_sb = qkv.tile([P, N_IT, D], BF16)      # v[p, jt, e] = v[jt*128+p, e]

        with tc.tile_pool(name="proj_phase", bufs=1) as pp:
            # ---- resident weights: w_sb[p, a, e] = w[a*128+p, e] ----
            # All on the SWDGE (gpsimd) queue: HWDGE copies serialize against
            # the xbar transposes (HW deadlock guard). Issued interleaved with
            # the first x chunk loads below.
            w_sbs = {}

            def load_weight(name, w_dram):
                w_sbs[name] = pp.tile([P, N_DB, D], BF16, tag=f"w_{name}", bufs=1,
                                      name=f"w_sb_{name}")
                nc.gpsimd.dma_start(out=w_sbs[name],
                                    in_=w_dram[:, :].rearrange("(a p) e -> p a e", p=P))

            x_chunks = {}

            def load_chunk(pname, x_dram, ic):
                """One SWDGE DMA (cast f32->bf16) for a whole 512-token chunk."""
                xc = pp.tile([P, CT, D], BF16, tag="x_chunk", bufs=3,
                             name=f"x_{pname}_{ic}")
                nc.gpsimd.dma_start(
                    out=xc,
                    in_=x_dram[ts(ic, 512), :].rearrange("(s p) d -> p s d", p=P),
                )
                x_chunks[(pname, ic)] = xc

            def ln_chunk(pname, xt_tile, ic):
                """Layernorm + transpose of one loaded chunk into xt_tile."""
                xc = x_chunks.pop((pname, ic))
                held = []
                for s in range(CT):
                    stats = pp.tile([P, 2, 6], F32, tag="stats", bufs=8,
                                    name=f"stats_{pname}_{ic}_{s}")
                    for sb in range(2):
                        nc.vector.bn_stats(out=stats[:, sb, :], in_=xc[:, s, ts(sb, 512)])
                    mv = pp.tile([P, 2], F32, tag="mv", bufs=8,
                                 name=f"mv_{pname}_{ic}_{s}")
                    nc.vector.bn_aggr(out=mv, in_=stats)
                    rstd = pp.tile([P, 1], F32, tag="rstd", bufs=8,
                                   name=f"rstd_{pname}_{ic}_{s}")
                    nc.scalar.activation(
                        out=rstd, in_=mv[:, 1:2],
                        func=mybir.ActivationFunctionType.Abs_reciprocal_sqrt,
                        bias=eps_t,
                    )
                    held.append((s, mv, rstd))
                for s, mv, rstd in held:
                    ln_out = pp.tile([P, D], BF16, tag="ln_out", bufs=4,
                                     name=f"ln_out_{pname}_{ic}_{s}")
                    nc.vector.tensor_scalar(
                        out=ln_out, in0=xc[:, s, :], scalar1=mv[:, 0:1], scalar2=rstd,
                        op0=mybir.AluOpType.subtract, op1=mybir.AluOpType.mult,
                    )
                    nc.sync.dma_start(out=xt_tile[:, s], in_=ln_out, transpose=True)

            def proj_chunkT(w_sb, bias_sb, xt_tile, dstT, pname, ic):
                """dstT[:, eb, ic*512:+512] = w^T @ ln_chunk + bias."""
                for eb in range(N_EB):
                    ps = mm_ps.tile([P, 512], F32, tag="mm", name=f"ps_{pname}_{eb}_{ic}")
                    for db in range(N_DB):
                        nc.tensor.matmul(
                            ps, lhsT=w_sb[:, db, ts(eb, P)],
                            rhs=xt_tile[:, :, db, :],
                            start=(db == 0), stop=(db == N_DB - 1),
                        )
                    nc.scalar.activation(
                        out=dstT[:, eb, ts(ic, 512)], in_=ps,
                        func=mybir.ActivationFunctionType.Identity,
                        bias=bias_sb[:, eb:eb + 1],
                    )

            def proj_chunkV(w_sb, xt_tile, ic):
                """v_sb[:, ic*4+s, :] = ln_chunk^T @ w + bias (natural layout)."""
                for s in range(CT):
                    jt = ic * CT + s
                    for ec in range(N_EC):
                        ps = mm_ps.tile([P, 512], F32, tag="mm",
                                        name=f"ps_v_{jt}_{ec}")
                        for db in range(N_DB):
                            nc.tensor.matmul(
                                ps,
                                lhsT=xt_tile[:, s, db, :],
                                rhs=w_sb[:, db, ts(ec, 512)],
                                start=(db == 0), stop=(db == N_DB - 1),
                            )
                        nc.vector.tensor_add(
                            out=v_sb[:, jt, ts(ec, 512)], in0=ps,
                            in1=bv_bc[:, ts(ec, 512)],
                        )

            # ---- three phases, each software-pipelined one chunk deep ----
            phases = [
                ("t", x_t, lambda xt, ic: proj_chunkT(w_sbs["q"], bq_sb, xt, qT, "q", ic)),
                ("k", x_k, lambda xt, ic: proj_chunkT(w_sbs["k"], bk_sb, xt, kT, "k", ic)),
                ("v", x_v, lambda xt, ic: proj_chunkV(w_sbs["v"], xt, ic)),
            ]
            # global chunk-load order, prefetched 2 ahead across phase bounds
            order = [(pn, xd, ic) for pn, xd, _ in phases for ic in range(N_IC)]
            n_loaded = [0]

            def ensure_loaded(upto):
                while n_loaded[0] <= min(upto, len(order) - 1):
                    pn, xd, ic = order[n_loaded[0]]
                    load_chunk(pn, xd, ic)
                    n_loaded[0] += 1

            # prologue: first chunk + wq are the critical path
            ensure_loaded(0)
            load_weight("q", wq)
            load_consts()
            ensure_loaded(1)
            load_weight("k", wk)
            load_weight("v", wv)

            g = 0
            for pname, x_dram, proj_fn in phases:
                xts = {}
                with nc.named_scope(f"phase_{pname}"):
                    for ic in range(N_IC + 1):
                        if ic < N_IC:
                            ensure_loaded(g + 2)
                            xts[ic] = pp.tile([P, CT, N_DB, P], BF16, tag="xt",
                                              bufs=3, name=f"xt_{pname}_{ic}")
                            ln_chunk(pname, xts[ic], ic)
                            g += 1
                        if ic > 0:
                            proj_fn(xts.pop(ic - 1), ic - 1)

        # ---- attention, software-pipelined one chunk deep ----
        attv_ps = ctx.enter_context(tc.tile_pool(name="attv_ps", bufs=2, space="PSUM"))
        with tc.tile_pool(name="att", bufs=1) as att:
            aTs = {}

            def scores_chunk(ic):
                aT = att.tile([P, N_IT, 512], BF16, tag="aT", bufs=2, name=f"aT_{ic}")
                aTs[ic] = aT
                with nc.named_scope(f"scores_{ic}"):
                    for jt in range(N_IT):
                        ps = mm_ps.tile([P, 512], F32, tag="mm", name=f"ps_s_{ic}_{jt}")
                        for eb in range(N_EB):
                            nc.tensor.matmul(
                                ps, lhsT=kT[:, eb, ts(jt, P)],
                                rhs=qT[:, eb, ts(ic, 512)],
                                start=(eb == 0), stop=(eb == N_EB - 1),
                            )
                        nc.scalar.activation(
                            out=aT[:, jt, :], in_=ps,
                            func=mybir.ActivationFunctionType.Exp, scale=SCALE,
                        )

            def attv_chunk(ic):
                aT = aTs.pop(ic)
                with nc.named_scope(f"attv_{ic}"):
                    for isub in range(4):
                        ou = attv_ps.tile([P, D], F32, tag="ou", name=f"ou_{ic}_{isub}")
                        zz = mm_ps.tile([P, 1], F32, tag="mm", name=f"z_{ic}_{isub}")
                        # same-bank runs of 16 accumulating matmuls (bank cycling
                        # between consecutive matmuls forces PE micro-stalls)
                        for ec in range(N_EC):
                            for jt in range(N_IT):
                                nc.tensor.matmul(
                                    ou[:, ts(ec, 512)], lhsT=aT[:, jt, ts(isub, P)],
                                    rhs=v_sb[:, jt, ts(ec, 512)],
                                    start=(jt == 0), stop=(jt == N_IT - 1))
                        for jt in range(N_IT):
                            nc.tensor.matmul(zz, lhsT=aT[:, jt, ts(isub, P)], rhs=ones_t,
                                             start=(jt == 0), stop=(jt == N_IT - 1))
                        rz = att.tile([P, 1], F32, tag="rz", bufs=2,
                                      name=f"rz_{ic}_{isub}")
                        nc.vector.reciprocal(out=rz, in_=zz)
                        o_sb = att.tile([P, D], F32, tag="o_sb", bufs=2,
                                        name=f"o_{ic}_{isub}")
                        nc.vector.tensor_scalar_mul(out=o_sb, in0=ou, scalar1=rz)
                        nc.sync.dma_start(out=out[ts(ic * 4 + isub, P), :], in_=o_sb)

            scores_chunk(0)
            for ic in range(N_IC):
                if ic + 1 < N_IC:
                    scores_chunk(ic + 1)
                attv_chunk(ic)

    nc.compile()
    return nc


_NC_CACHE = None


def _get_module():
    global _NC_CACHE
    if _NC_CACHE is None:
        _NC_CACHE = build_module()
    return _NC_CACHE


def kernel(target, source_k, source_v, Wq, bq, Wk, bk, Wv, bv,
           g_t, b_t, g_k, b_k, g_v, b_v):
    target = np.asarray(target, dtype=np.float32)
    source_k = np.asarray(source_k, dtype=np.float32)
    source_v = np.asarray(source_v, dtype=np.float32)
    Wq = np.asarray(Wq, dtype=np.float32); bq = np.asarray(bq, dtype=np.float32)
    Wk = np.asarray(Wk, dtype=np.float32); bk = np.asarray(bk, dtype=np.float32)
    Wv = np.asarray(Wv, dtype=np.float32); bv = np.asarray(bv, dtype=np.float32)
    g_t = np.asarray(g_t, dtype=np.float32); b_t = np.asarray(b_t, dtype=np.float32)
    g_k = np.asarray(g_k, dtype=np.float32); b_k = np.asarray(b_k, dtype=np.float32)
    g_v = np.asarray(g_v, dtype=np.float32); b_v = np.asarray(b_v, dtype=np.float32)

    bf16 = ml_dtypes.bfloat16
    # Fold the layernorm affine (g, b) into the projection weights/biases:
    #   LN_affine(x) @ W.T + b  ==  LN_plain(x) @ (W*g).T + (b + W @ b_ln)
    wqT = np.ascontiguousarray((Wq * g_t[None, :]).T).astype(bf16)
    wkT = np.ascontiguousarray((Wk * g_k[None, :]).T).astype(bf16)
    wvT = np.ascontiguousarray((Wv * g_v[None, :]).T).astype(bf16)
    bq_f = bq + Wq @ b_t
    bk_f = bk + Wk @ b_k
    bv_f = bv + Wv @ b_v

    nc = _get_module()
    in_maps = []
    for b in range(B):
        in_maps.append({
            "x_t": np.ascontiguousarray(target[b]),
            "x_k": np.ascontiguousarray(source_k[b]),
            "x_v": np.ascontiguousarray(source_v[b]),
            "wq": wqT, "wk": wkT, "wv": wvT,
            "bq": bq_f, "bk": bk_f, "bv": bv_f,
        })

    res = run_bass_kernel_spmd(nc, in_maps, core_ids=list(range(B)),
                               trace=bool(int(os.environ.get("KERNEL_TRACE", "0"))))
    out = np.stack([res.results[b]["out"] for b in range(B)], axis=0)
    kernel.last_results = res
    return out
